# revision 7
# baseline (speedup 1.0000x reference)
"""Canny edge detector on 8 Trainium2 NeuronCores (Bass/Tile).

Strategy (pure data parallelism, one 3x1024x1024 image per core):
  - Image split into 9 row-strips of 128 partitions (118 interior rows +
    5-row halo each side); 8-column zero margins in the free axis.
  - All vertical convolutions run on the TensorEngine as banded-matrix
    matmuls (gauss5*[1,2,1] and gauss5*[1,0,-1] composed 7-tap operators,
    row shifts for NMS, tridiagonal for hysteresis connectivity).
  - Horizontal taps + all nonlinear work run on DVE/GPSIMD/ACT with fused
    custom DVE micro-ops (orientation classification by tan comparisons
    instead of atan2; NMS as mag > max(opposite-neighbor pair)).
"""
import math

import numpy as np

import concourse.bacc as bacc
import concourse.bass as bass
import concourse.tile as tile
import concourse.mybir as mybir
from concourse import bass_utils
from concourse.dve_spec import Spec, Src0, Src1, C0, C1, Zero, sq, maxx, lower
from concourse.dve_uop import DveOpSpec
import concourse.dve_ops as dve_ops
from concourse.dve_ops import DveOp, OPS

AOP = mybir.AluOpType
AF = mybir.ActivationFunctionType
F32 = mybir.dt.float32
U8 = mybir.dt.uint8

H = W = 1024
NS = 9          # strips
IH = 118        # interior rows per strip
HALO = 5        # rows of halo above/below
LM = 8          # left/right zero margin columns
FW = W + 2 * LM # tile free width

T1 = math.tan(math.radians(22.5))
T2 = math.tan(math.radians(67.5))
THR_LO, THR_HI = 10.0, 100.0


# --------------------------- custom DVE ops ---------------------------------
def _register(name, spec):
    for o in OPS:
        if o.name == name:
            return o
    shas = {}
    for ver in ("v3", "v4"):
        s = DveOpSpec(name=name, opcode=0, uops=lower(spec, ver=ver))
        shas[ver] = s.sha(ver)
    op = DveOp(name, spec, subdim=False, uops_sha=shas)
    OPS.append(op)
    dve_ops._SUB_OPCODE_FOR_NAME[name] = dve_ops._CUSTOM_DVE_ROW_BASE + len(OPS) - 1
    dve_ops.CUSTOM_DVE_SPECS[name] = spec
    return op


OP_AB2 = _register("CANNY_AB2", Spec(
    body=(Src0 + Src1) * C0,
    reference=lambda in0, in1, s0, s1, imm2: ((in0 + in1) * s0).astype(np.float32)))
OP_SQ2 = _register("CANNY_SQ2", Spec(
    body=sq(Src0) + sq(Src1),
    reference=lambda in0, in1, s0, s1, imm2: (in0 * in0 + in1 * in1).astype(np.float32)))
OP_MH = _register("CANNY_MH", Spec(
    body=(maxx(Src0, -Src0) * C0) >= maxx(Src1, -Src1),
    reference=lambda in0, in1, s0, s1, imm2:
        (np.abs(in0) * s0 >= np.abs(in1)).astype(np.float32)))
OP_MV = _register("CANNY_MV", Spec(
    body=(maxx(Src0, -Src0) * C0) < maxx(Src1, -Src1),
    reference=lambda in0, in1, s0, s1, imm2:
        (np.abs(in0) * s0 < np.abs(in1)).astype(np.float32)))
OP_SD = _register("CANNY_SD", Spec(
    body=(Src0 * Src1) > Zero,
    reference=lambda in0, in1, s0, s1, imm2: (in0 * in1 > 0).astype(np.float32)))
OP_HI = _register("CANNY_HI", Spec(
    body=(Src0 > Src1) * (Src0 > C0),
    reference=lambda in0, in1, s0, s1, imm2:
        ((in0 > in1) & (in0 > s0)).astype(np.float32)))
OP_MID = _register("CANNY_MID", Spec(
    body=(Src0 > Src1) * ((Src0 >= C0) - (Src0 > C1)),
    reference=lambda in0, in1, s0, s1, imm2:
        ((in0 > in1) & (in0 >= s0) & ~(in0 > s1)).astype(np.float32)))


# --------------------------- constant matrices -------------------------------
def build_mats():
    """[5,128,128]: V1, V2 (7-tap vertical ops), row-shift up/down, tridiag."""
    g = np.exp(-0.5 * (np.arange(5) - 2.0) ** 2).astype(np.float32)
    V1 = np.zeros(7, np.float32)
    V2 = np.zeros(7, np.float32)
    for d1 in range(-2, 3):
        for d2, w in zip((-1, 0, 1), (1.0, 2.0, 1.0)):
            V1[d1 + d2 + 3] += g[d1 + 2] * np.float32(w)
        V2[d1 - 1 + 3] += g[d1 + 2]
        V2[d1 + 1 + 3] -= g[d1 + 2]
    mats = np.zeros((5, 128, 128), np.float32)
    k = np.arange(128)[:, None]
    m = np.arange(128)[None, :]
    d = k - m
    for dd in range(-3, 4):
        mats[0][d == dd] = V1[dd + 3]
        mats[1][d == dd] = V2[dd + 3]
    mats[2][d == -1] = 1.0  # ab[m] = in[m-1]  (row above)
    mats[3][d == 1] = 1.0   # be[m] = in[m+1]  (row below)
    for dd in (-1, 0, 1):
        mats[4][d == dd] = 1.0  # tridiagonal ones
    return mats


# --------------------------- the Bass program --------------------------------
def build_nc():
    g = np.exp(-0.5 * (np.arange(5) - 2.0) ** 2).astype(np.float32)
    g0, g1 = float(g[0]), float(g[1])

    nc = bacc.Bacc("TRN2", target_bir_lowering=False, debug=False, num_devices=8)
    img_d = nc.dram_tensor("img3", [3, H, W], F32, kind="ExternalInput")
    mats_d = nc.dram_tensor("mats", [5, 128, 128], F32, kind="ExternalInput")
    out_d = nc.dram_tensor("edge", [H, W], U8, kind="ExternalOutput")

    with tile.TileContext(nc) as tc:
        with (
            tc.tile_pool(name="consts", bufs=1) as consts,
            tc.tile_pool(name="xin", bufs=4) as xin,
            tc.tile_pool(name="work", bufs=2) as work,
            tc.tile_pool(name="nms", bufs=1) as nms,
            tc.tile_pool(name="psA", bufs=1, space="PSUM") as psA,
            tc.tile_pool(name="psB", bufs=1, space="PSUM") as psB,
        ):
            m_v1 = consts.tile([128, 128], F32, tag="m_v1")
            m_v2 = consts.tile([128, 128], F32, tag="m_v2")
            m_ab = consts.tile([128, 128], F32, tag="m_ab")
            m_be = consts.tile([128, 128], F32, tag="m_be")
            m_t3 = consts.tile([128, 128], F32, tag="m_t3")
            for i, t in enumerate((m_v1, m_v2, m_ab, m_be, m_t3)):
                nc.sync.dma_start(out=t, in_=mats_d.ap()[i])

            for s in range(NS):
                ytop = IH * s - HALO            # y of partition 0
                y0 = max(0, ytop)
                y1 = min(H, ytop + 128)
                p0 = y0 - ytop
                p1 = y1 - ytop

                # ---- magnitude accumulator & per-strip planes ----
                mag = nms.tile([128, FW], F32, tag="mag")
                nc.vector.memset(mag[:, 0:LM], 0.0)
                nc.vector.memset(mag[:, W + LM:FW], 0.0)

                gxs_ps = psB.tile([128, W], F32, tag="gxs")
                gys_ps = psB.tile([128, W], F32, tag="gys")

                for c in range(3):
                    x = xin.tile([128, FW], F32, tag="x")
                    nc.vector.memset(x[:, 0:LM], 0.0)
                    nc.vector.memset(x[:, W + LM:FW], 0.0)
                    if p0 > 0:
                        nc.vector.memset(x[0:32 * ((p0 + 31) // 32), :], 0.0)
                    if p1 < 128:
                        nc.vector.memset(x[32 * (p1 // 32):128, :], 0.0)
                    nc.sync.dma_start(out=x[p0:p1, LM:W + LM],
                                      in_=img_d.ap()[c, y0:y1, :])

                    # horizontal gaussian blur (5 taps, center weight 1)
                    t1t = work.tile([128, FW], F32, tag="t1")
                    t2t = work.tile([128, FW], F32, tag="t2")
                    hb = work.tile([128, FW], F32, tag="hb")
                    n = FW - 4
                    nc.gpsimd.tensor_tensor(out=t1t[:, 2:FW - 2], in0=x[:, 1:FW - 3],
                                            in1=x[:, 3:FW - 1], op=AOP.add)
                    nc.vector._custom_dve(OP_AB2, out=t2t[:, 2:FW - 2],
                                          in0=x[:, 0:FW - 4], in1=x[:, 4:FW],
                                          s0=g0)
                    nc.vector.scalar_tensor_tensor(out=t1t[:, 2:FW - 2],
                                                   in0=t1t[:, 2:FW - 2], scalar=g1,
                                                   in1=t2t[:, 2:FW - 2],
                                                   op0=AOP.mult, op1=AOP.add)
                    nc.gpsimd.tensor_tensor(out=hb[:, 2:FW - 2], in0=t1t[:, 2:FW - 2],
                                            in1=x[:, 2:FW - 2], op=AOP.add)

                    # horizontal sobel components
                    d_t = work.tile([128, FW], F32, tag="d")
                    e_t = work.tile([128, FW], F32, tag="e")
                    s_t = work.tile([128, FW], F32, tag="s")
                    nc.vector.tensor_tensor(out=d_t[:, 3:FW - 3], in0=hb[:, 2:FW - 4],
                                            in1=hb[:, 4:FW - 2], op=AOP.subtract)
                    nc.gpsimd.tensor_tensor(out=e_t[:, 3:FW - 3], in0=hb[:, 2:FW - 4],
                                            in1=hb[:, 4:FW - 2], op=AOP.add)
                    h2 = work.tile([128, FW], F32, tag="h2")
                    nc.gpsimd.tensor_tensor(out=h2[:, 3:FW - 3], in0=hb[:, 3:FW - 3],
                                            in1=hb[:, 3:FW - 3], op=AOP.add)
                    nc.gpsimd.tensor_tensor(out=s_t[:, 3:FW - 3], in0=h2[:, 3:FW - 3],
                                            in1=e_t[:, 3:FW - 3], op=AOP.add)

                    # vertical 7-tap operators on PE; accumulate channel sums
                    gx_ps = psA.tile([128, W], F32, tag="pa")
                    gy_ps = psA.tile([128, W], F32, tag="pb")
                    for h0 in (0, 512):
                        rhs = d_t[:, LM + h0:LM + h0 + 512]
                        nc.tensor.matmul(out=gx_ps[:, h0:h0 + 512], lhsT=m_v1,
                                         rhs=rhs, start=True, stop=True)
                        nc.tensor.matmul(out=gxs_ps[:, h0:h0 + 512], lhsT=m_v1,
                                         rhs=rhs, start=(c == 0), stop=(c == 2))
                        rhs = s_t[:, LM + h0:LM + h0 + 512]
                        nc.tensor.matmul(out=gy_ps[:, h0:h0 + 512], lhsT=m_v2,
                                         rhs=rhs, start=True, stop=True)
                        nc.tensor.matmul(out=gys_ps[:, h0:h0 + 512], lhsT=m_v2,
                                         rhs=rhs, start=(c == 0), stop=(c == 2))

                    gy_sb = work.tile([128, W], F32, tag="gy")
                    nc.scalar.copy(out=gy_sb, in_=gy_ps)
                    q = work.tile([128, W], F32, tag="q")
                    nc.vector._custom_dve(OP_SQ2, out=q, in0=gx_ps, in1=gy_sb)
                    if c == 0:
                        nc.scalar.activation(out=mag[:, LM:W + LM], in_=q, func=AF.Sqrt)
                    else:
                        sc = work.tile([128, W], F32, tag="sc")
                        nc.scalar.activation(out=sc, in_=q, func=AF.Sqrt)
                        nc.vector.tensor_tensor(out=mag[:, LM:W + LM],
                                                in0=mag[:, LM:W + LM], in1=sc,
                                                op=AOP.add)

                # ---- orientation classification ----
                gys_sb = nms.tile([128, W], F32, tag="gys_sb")
                nc.scalar.copy(out=gys_sb, in_=gys_ps)
                mh = nms.tile([128, W], U8, tag="mh")
                mv = nms.tile([128, W], U8, tag="mv")
                sd = nms.tile([128, W], U8, tag="sd")
                nc.vector._custom_dve(OP_MH, out=mh, in0=gxs_ps, in1=gys_sb, s0=T1)
                nc.vector._custom_dve(OP_MV, out=mv, in0=gxs_ps, in1=gys_sb, s0=T2)
                nc.vector._custom_dve(OP_SD, out=sd, in0=gxs_ps, in1=gys_sb)

                # ---- NMS: row-shifted mags via PE, pair maxes, select ----
                ab_ps = psA.tile([128, W], F32, tag="pa")  # mag[y-1]
                be_ps = psA.tile([128, W], F32, tag="pb")  # mag[y+1]
                for h0 in (0, 512):
                    rhs = mag[:, LM + h0:LM + h0 + 512]
                    nc.tensor.matmul(out=ab_ps[:, h0:h0 + 512], lhsT=m_ab,
                                     rhs=rhs, start=True, stop=True)
                    nc.tensor.matmul(out=be_ps[:, h0:h0 + 512], lhsT=m_be,
                                     rhs=rhs, start=True, stop=True)
                ab_sb = nms.tile([128, W], F32, tag="ab_sb")
                nc.scalar.copy(out=ab_sb, in_=ab_ps)

                sel = nms.tile([128, W], F32, tag="sel")
                p1t = nms.tile([128, W], F32, tag="p1t")
                p02 = nms.tile([128, W], F32, tag="p02")
                # P3 = max(ab[x+1], be[x-1]) -> sel base
                nc.vector.tensor_tensor(out=sel[:, 1:W - 1], in0=ab_sb[:, 2:W],
                                        in1=be_ps[:, 0:W - 2], op=AOP.max)
                nc.vector.tensor_copy(out=sel[:, 0:1], in_=ab_sb[:, 1:2])
                nc.vector.tensor_copy(out=sel[:, W - 1:W], in_=be_ps[:, W - 2:W - 1])
                # P1 = max(ab[x-1], be[x+1])
                nc.vector.tensor_tensor(out=p1t[:, 1:W - 1], in0=ab_sb[:, 0:W - 2],
                                        in1=be_ps[:, 2:W], op=AOP.max)
                nc.vector.tensor_copy(out=p1t[:, 0:1], in_=be_ps[:, 1:2])
                nc.vector.tensor_copy(out=p1t[:, W - 1:W], in_=ab_sb[:, W - 2:W - 1])
                nc.vector.copy_predicated(out=sel, mask=sd, data=p1t)
                # P2 = max(ab, be)
                nc.vector.tensor_tensor(out=p02, in0=ab_sb, in1=be_ps, op=AOP.max)
                nc.vector.copy_predicated(out=sel, mask=mv, data=p02)
                # P0 = max(mag[x-1], mag[x+1])
                nc.vector.tensor_tensor(out=p02, in0=mag[:, LM - 1:W + LM - 1],
                                        in1=mag[:, LM + 1:W + LM + 1], op=AOP.max)
                nc.vector.copy_predicated(out=sel, mask=mh, data=p02)

                # ---- thresholds ----
                higher = nms.tile([128, FW], F32, tag="higher")
                nc.vector.memset(higher[:, 0:LM], 0.0)
                nc.vector.memset(higher[:, W + LM:FW], 0.0)
                midm = nms.tile([128, W], F32, tag="midm")
                nc.vector._custom_dve(OP_HI, out=higher[:, LM:W + LM],
                                      in0=mag[:, LM:W + LM], in1=sel, s0=THR_HI)
                nc.vector._custom_dve(OP_MID, out=midm,
                                      in0=mag[:, LM:W + LM], in1=sel,
                                      s0=THR_LO, s1=THR_HI)

                # ---- hysteresis connectivity: 3x3 ones via PE accumulation ----
                s3_ps = psA.tile([128, W], F32, tag="pa")
                for h0 in (0, 512):
                    for j, dx in enumerate((-1, 0, 1)):
                        rhs = higher[:, LM + h0 + dx:LM + h0 + dx + 512]
                        nc.tensor.matmul(out=s3_ps[:, h0:h0 + 512], lhsT=m_t3,
                                         rhs=rhs, start=(j == 0), stop=(j == 2))
                cm = nms.tile([128, W], F32, tag="cm")
                nc.vector.tensor_tensor(out=cm, in0=s3_ps, in1=higher[:, LM:W + LM],
                                        op=AOP.is_gt)
                nc.gpsimd.tensor_tensor(out=cm, in0=cm, in1=midm, op=AOP.mult)
                nc.vector.tensor_tensor(out=higher[:, LM:W + LM],
                                        in0=higher[:, LM:W + LM], in1=cm, op=AOP.max)

                # ---- cast + store interior rows (borders stay zero) ----
                u8t = nms.tile([128, W], U8, tag="u8t")
                nc.vector.tensor_copy(out=u8t, in_=higher[:, LM:W + LM])
                nc.vector.memset(u8t[:, 0:1], 0)
                nc.vector.memset(u8t[:, W - 1:W], 0)
                oy0 = max(1, IH * s)
                oy1 = min(H - 1, IH * s + IH)
                if oy1 > oy0:
                    q0 = oy0 - ytop
                    q1 = oy1 - ytop
                    nc.sync.dma_start(out=out_d.ap()[oy0:oy1, :], in_=u8t[q0:q1, :])

    nc.compile()
    return nc


_NC_CACHE = None


def _get_nc():
    global _NC_CACHE
    if _NC_CACHE is None:
        _NC_CACHE = build_nc()
    return _NC_CACHE


def kernel(img, gauss_h=None, gauss_v=None, sobel_h=None, sobel_v=None,
           dir_filt=None, conn_filt=None, **_unused):
    img = np.asarray(img, dtype=np.float32)
    B = img.shape[0]
    assert img.shape == (B, 3, H, W)
    mats = build_mats()
    nc = _get_nc()
    in_maps = [{"img3": np.ascontiguousarray(img[b]), "mats": mats}
               for b in range(B)]
    res = bass_utils.run_bass_kernel_spmd(nc, in_maps, core_ids=list(range(B)))
    out = np.stack([res.results[b]["edge"] for b in range(B)])
    return out.astype(np.uint8)


if __name__ == "__main__":
    rng = np.random.RandomState(0)
    img = (rng.rand(8, 3, H, W) * 255).astype(np.float32)
    e = kernel(img)
    print("kernel ran; edge fraction:", e.mean())
